# revision 2
# baseline (speedup 1.0000x reference)
"""DMVFlow per-state diagonal-Gaussian log-density kernel for 8 TRN2 NeuronCores.

density[b,t,k] = log_norm - 0.5*(s2[b,t] - 2*cross[b,t,k] + m2[k])
  with  log_norm = -0.5*(D*log(2pi) + sum_d log var[d])
        s2[b,t]  = sum_d s[b,t,d]^2 / var[d]
        cross    = sum_d s[b,t,d] * means[k,d] / var[d]
        m2[k]    = sum_d means[k,d]^2 / var[d]

Sharding: data-parallel over batch (32 sentences per core), means/var replicated.

Device program (per core, rows = 32*256 = 8192 token positions):
  - s arrives pre-transposed on host as st[c, p, n] = s[n, 128*c + p] in fp16
    (contraction dim d = 128*c + p on SBUF partitions; n = token row).
  - PSUM[k, n] accumulates 12 matmuls per 512-row tile:
      6 x (W chunk)     : W[d,k] = means[k,d]/var[d]      -> cross term
      6 x (V chunk)     : V[d,k] = -0.5/var[d] (repl. k)  -> -0.5*s2 term,
                          moving operand = s^2 (squared on ACT/DVE engines)
  - out[k, n] = PSUM + c[k],  c[k] = log_norm - 0.5*m2[k]  (DVE tensor_scalar)
  - Host transposes the (128, 8192) per-core result back to (32, 256, 128).
"""

import numpy as np

N_CORES = 8
B, T, D, K = 256, 256, 768, 128
BPC = B // N_CORES          # batches per core
R = BPC * T                 # rows (token positions) per core = 8192
TN = 512                    # rows per tile
NT = R // TN                # tiles per core = 16
C = D // 128                # contraction chunks = 6

_NC = None                  # cached bass program (build once per process)


def _build_nc():
    from contextlib import ExitStack

    import concourse.bacc as bacc
    import concourse.tile as tile
    from concourse import mybir

    f16 = mybir.dt.float16
    f32 = mybir.dt.float32

    nc = bacc.Bacc(None, target_bir_lowering=False, debug=False)

    st = nc.dram_tensor("st", [C, 128, R], f16, kind="ExternalInput")
    wt = nc.dram_tensor("wt", [128, C, K], f16, kind="ExternalInput")
    vt = nc.dram_tensor("vt", [128, C, K], f16, kind="ExternalInput")
    cv = nc.dram_tensor("cv", [K, 1], f32, kind="ExternalInput")
    out = nc.dram_tensor("out", [K, R], f32, kind="ExternalOutput")

    with tile.TileContext(nc) as tc, ExitStack() as ctx:
        consts = ctx.enter_context(tc.tile_pool(name="consts", bufs=1))
        inp = ctx.enter_context(tc.tile_pool(name="inp", bufs=4))
        sq = ctx.enter_context(tc.tile_pool(name="sq", bufs=4))
        outp = ctx.enter_context(tc.tile_pool(name="outp", bufs=4))
        psum = ctx.enter_context(
            tc.tile_pool(name="psum", bufs=8, space="PSUM")
        )

        w_sb = consts.tile([128, C, K], f16)
        nc.sync.dma_start(w_sb[:], wt[:])
        v_sb = consts.tile([128, C, K], f16)
        nc.sync.dma_start(v_sb[:], vt[:])
        c_sb = consts.tile([K, 1], f32)
        nc.sync.dma_start(c_sb[:], cv[:])

        for t in range(NT):
            n0 = t * TN
            s_t = inp.tile([128, C, TN], f16, tag="s")
            nc.sync.dma_start(
                s_t[:], st[:, :, n0 : n0 + TN].rearrange("c p n -> p c n")
            )

            # squares: ACT takes 4 chunks, DVE takes 2 (split keeps both busy
            # under PE's 12-matmul tile time)
            sq_a = sq.tile([128, 4, TN], f16, tag="sqa")
            nc.scalar.activation(
                sq_a[:], s_t[:, 0:4, :], mybir.ActivationFunctionType.Square
            )
            sq_d = sq.tile([128, 2, TN], f16, tag="sqd")
            nc.vector.tensor_mul(sq_d[:], s_t[:, 4:6, :], s_t[:, 4:6, :])

            acc = psum.tile([K, TN], f32)
            for c in range(C):
                nc.tensor.matmul(
                    acc[:], w_sb[:, c, :], s_t[:, c, :],
                    start=(c == 0), stop=False,
                )
            for c in range(4):
                nc.tensor.matmul(
                    acc[:], v_sb[:, c, :], sq_a[:, c, :],
                    start=False, stop=False,
                )
            for c in range(2):
                nc.tensor.matmul(
                    acc[:], v_sb[:, 4 + c, :], sq_d[:, c, :],
                    start=False, stop=(c == 1),
                )

            o_t = outp.tile([K, TN], f32, tag="o")
            nc.vector.tensor_scalar_add(o_t[:], acc[:], c_sb[:])
            nc.sync.dma_start(out[:, n0 : n0 + TN], o_t[:])

    return nc


def _get_nc():
    global _NC
    if _NC is None:
        _NC = _build_nc()
        _NC.finalize()  # Bacc.finalize -> compile (reg alloc etc.) + freeze
    return _NC


def prep_in_maps(s, means, var):
    s = np.asarray(s)
    means = np.asarray(means, dtype=np.float64)
    var = np.asarray(var, dtype=np.float64)

    inv = 1.0 / var
    w = np.ascontiguousarray(
        (means * inv[None, :]).T.reshape(C, 128, K).transpose(1, 0, 2)
    ).astype(np.float16)                                   # [p, c, k]
    v = np.ascontiguousarray(
        np.broadcast_to(
            (-0.5 * inv).reshape(C, 128, 1).transpose(1, 0, 2), (128, C, K)
        )
    ).astype(np.float16)                                   # [p, c, k]
    log_norm = -0.5 * (D * np.log(2.0 * np.pi) + np.sum(np.log(var)))
    m2 = (means * means) @ inv                             # (K,)
    cvec = (log_norm - 0.5 * m2).astype(np.float32).reshape(K, 1)

    s16 = s.astype(np.float16).reshape(N_CORES, R, D)
    in_maps = []
    for i in range(N_CORES):
        st_i = np.ascontiguousarray(s16[i].T).reshape(C, 128, R)
        in_maps.append({"st": st_i, "wt": w, "vt": v, "cv": cvec})
    return in_maps


def run_device(in_maps, trace=False, trace_kwargs=None):
    from concourse.bass_utils import run_bass_kernel_spmd

    return run_bass_kernel_spmd(
        _get_nc(),
        in_maps,
        list(range(N_CORES)),
        trace=trace,
        **(trace_kwargs or {}),
    )


def assemble(results):
    full = np.empty((B, T, K), dtype=np.float32)
    for i in range(N_CORES):
        o = np.asarray(results[i]["out"])                  # (K, R)
        full[i * BPC : (i + 1) * BPC] = o.T.reshape(BPC, T, K)
    return full


def kernel(s, means, var):
    in_maps = prep_in_maps(s, means, var)
    br = run_device(in_maps)
    return assemble(br.results)


# revision 5
# speedup vs baseline: 1.1177x; 1.1177x over previous
"""DMVFlow per-state diagonal-Gaussian log-density kernel for 8 TRN2 NeuronCores.

density[b,t,k] = log_norm - 0.5*(s2[b,t] - 2*cross[b,t,k] + m2[k])
  with  log_norm = -0.5*(D*log(2pi) + sum_d log var[d])
        s2[b,t]  = sum_d s[b,t,d]^2 / var[d]
        cross    = sum_d s[b,t,d] * means[k,d] / var[d]
        m2[k]    = sum_d means[k,d]^2 / var[d]

Sharding: data-parallel over batch (32 sentences per core), means/var replicated.

Device program (per core, rows = 32*256 = 8192 token positions):
  - s arrives pre-transposed on host as st[c, p, n] = s[n, 128*c + p] in fp16
    (contraction dim d = 128*c + p on SBUF partitions; n = token row).
  - PSUM[k, n] accumulates 12 matmuls per 512-row tile:
      6 x (W chunk)     : W[d,k] = means[k,d]/var[d]      -> cross term
      6 x (V chunk)     : V[d,k] = -0.5/var[d] (repl. k)  -> -0.5*s2 term,
                          moving operand = s^2 (squared on ACT/DVE engines)
  - out[k, n] = PSUM + c[k],  c[k] = log_norm - 0.5*m2[k]  (DVE tensor_scalar)
  - Host transposes the (128, 8192) per-core result back to (32, 256, 128).
"""

import numpy as np

N_CORES = 8
B, T, D, K = 256, 256, 768, 128
BPC = B // N_CORES          # batches per core
R = BPC * T                 # rows (token positions) per core = 8192
TN = 512                    # rows per tile
NT = R // TN                # tiles per core = 16
C = D // 128                # contraction chunks = 6

_NC = None                  # cached bass program (build once per process)


def _build_nc():
    from contextlib import ExitStack

    import concourse.bacc as bacc
    import concourse.tile as tile
    from concourse import mybir

    f16 = mybir.dt.float16
    f32 = mybir.dt.float32

    nc = bacc.Bacc(None, target_bir_lowering=False, debug=False)

    st = nc.dram_tensor("st", [C, 128, R], f16, kind="ExternalInput")
    wt = nc.dram_tensor("wt", [128, C, K], f16, kind="ExternalInput")
    vt = nc.dram_tensor("vt", [128, C, K], f16, kind="ExternalInput")
    cv = nc.dram_tensor("cv", [K, 1], f32, kind="ExternalInput")
    out = nc.dram_tensor("out", [K, R], f32, kind="ExternalOutput")

    with tile.TileContext(nc) as tc, ExitStack() as ctx:
        consts = ctx.enter_context(tc.tile_pool(name="consts", bufs=1))
        inp = ctx.enter_context(tc.tile_pool(name="inp", bufs=10))
        sq = ctx.enter_context(tc.tile_pool(name="sq", bufs=6))
        outp = ctx.enter_context(tc.tile_pool(name="outp", bufs=6))
        psum = ctx.enter_context(
            tc.tile_pool(name="psum", bufs=8, space="PSUM")
        )

        w_sb = consts.tile([128, C, K], f16)
        nc.sync.dma_start(w_sb[:], wt[:])
        v_sb = consts.tile([128, C, K], f16)
        nc.sync.dma_start(v_sb[:], vt[:])
        c_sb = consts.tile([K, 1], f32)
        nc.sync.dma_start(c_sb[:], cv[:])

        for t in range(NT):
            n0 = t * TN
            s_t = inp.tile([128, C, TN], f16, tag="s")
            if t == 0:
                # chunk-granular first load: matmul c starts once chunk c lands
                for c in range(C):
                    nc.sync.dma_start(
                        s_t[:, c, :], st[c, :, n0 : n0 + TN]
                    )
            else:
                nc.sync.dma_start(
                    s_t[:], st[:, :, n0 : n0 + TN].rearrange("c p n -> p c n")
                )

            # squares: ACT takes 4 chunks, DVE takes 2 (split keeps both busy
            # under PE's 12-matmul tile time)
            sq_a = sq.tile([128, 4, TN], f16, tag="sqa")
            nc.scalar.activation(
                sq_a[:], s_t[:, 0:4, :], mybir.ActivationFunctionType.Square
            )
            sq_d = sq.tile([128, 2, TN], f16, tag="sqd")
            nc.vector.tensor_mul(sq_d[:], s_t[:, 4:6, :], s_t[:, 4:6, :])

            acc = psum.tile([K, TN], f32)
            for c in range(C):
                nc.tensor.matmul(
                    acc[:], w_sb[:, c, :], s_t[:, c, :],
                    start=(c == 0), stop=False,
                )
            for c in range(4):
                nc.tensor.matmul(
                    acc[:], v_sb[:, c, :], sq_a[:, c, :],
                    start=False, stop=False,
                )
            for c in range(2):
                nc.tensor.matmul(
                    acc[:], v_sb[:, 4 + c, :], sq_d[:, c, :],
                    start=False, stop=(c == 1),
                )

            o_t = outp.tile([K, TN], f32, tag="o")
            nc.vector.tensor_scalar_add(o_t[:], acc[:], c_sb[:])
            # scalar-engine HWDGE ring: keeps stores off the input DMA ring
            nc.scalar.dma_start(out[:, n0 : n0 + TN], o_t[:])

    return nc


def _get_nc():
    global _NC
    if _NC is None:
        _NC = _build_nc()
        _NC.finalize()  # Bacc.finalize -> compile (reg alloc etc.) + freeze
    return _NC


def prep_in_maps(s, means, var):
    s = np.asarray(s)
    means = np.asarray(means, dtype=np.float64)
    var = np.asarray(var, dtype=np.float64)

    inv = 1.0 / var
    w = np.ascontiguousarray(
        (means * inv[None, :]).T.reshape(C, 128, K).transpose(1, 0, 2)
    ).astype(np.float16)                                   # [p, c, k]
    v = np.ascontiguousarray(
        np.broadcast_to(
            (-0.5 * inv).reshape(C, 128, 1).transpose(1, 0, 2), (128, C, K)
        )
    ).astype(np.float16)                                   # [p, c, k]
    log_norm = -0.5 * (D * np.log(2.0 * np.pi) + np.sum(np.log(var)))
    m2 = (means * means) @ inv                             # (K,)
    cvec = (log_norm - 0.5 * m2).astype(np.float32).reshape(K, 1)

    s16 = s.astype(np.float16).reshape(N_CORES, R, D)
    in_maps = []
    for i in range(N_CORES):
        st_i = np.ascontiguousarray(s16[i].T).reshape(C, 128, R)
        in_maps.append({"st": st_i, "wt": w, "vt": v, "cv": cvec})
    return in_maps


def run_device(in_maps, trace=False, trace_kwargs=None):
    from concourse.bass_utils import run_bass_kernel_spmd

    return run_bass_kernel_spmd(
        _get_nc(),
        in_maps,
        list(range(N_CORES)),
        trace=trace,
        **(trace_kwargs or {}),
    )


def assemble(results):
    full = np.empty((B, T, K), dtype=np.float32)
    for i in range(N_CORES):
        o = np.asarray(results[i]["out"])                  # (K, R)
        full[i * BPC : (i + 1) * BPC] = o.T.reshape(BPC, T, K)
    return full


def kernel(s, means, var):
    in_maps = prep_in_maps(s, means, var)
    br = run_device(in_maps)
    return assemble(br.results)
